# revision 3
# baseline (speedup 1.0000x reference)
"""Multi-head attention kernel for Trainium2 (Bass/Tile), 8 NeuronCores.

Problem: q,k,v [16, 4096, 128] fp32 -> softmax(q@k^T/sqrt(128))@v.
Sharding: BH=16 heads split 2-per-core across 8 cores (head parallel, no
cross-core comms).

Per-head dataflow (n = query index, m = key index, d = head dim = 128):
  - PE-transpose Q,K into [d, n] / [d, m] SBUF layout (fp32, exact).
  - mm1: S^T chunk [m_chunk=128, n_tile=512] = KT_chunk.T @ QT_slice using
    float32r (1 cyc/row at N>=512, ~1.5e-4 rel).
  - ACT: exp(scale*S^T) PSUM->SBUF, fp16 out (scale folded into activation).
  - mm2: for each 128-query subtile, accumulate over all 32 m-chunks:
    psum[n_sub=128, 129] += expT_chunk(stationary) @ [V|1](moving, fp16).
    Columns 0:128 = unnormalized O, column 128 = softmax denominator.
  - DVE: reciprocal of col 128, tensor_scalar multiply -> O tile, batched
    2MB DMA store per head.
"""
import sys

sys.path.insert(0, "/opt/trn_rl_repo")

from contextlib import ExitStack

import numpy as np

import concourse.bass as bass
import concourse.mybir as mybir
import concourse.tile as tile
from concourse import bacc
from concourse.bass_utils import run_bass_kernel_spmd
from concourse.masks import make_identity

N_CORES = 8
H_PER_CORE = 2  # BH=16 / 8 cores
N = 4096  # sequence length
D = 128  # head dim
SCALE = float(D) ** -0.5

NT = N // 128  # 32 row tiles of 128
N_TILE = 512  # query tile width for mm1
N_NTILES = N // N_TILE  # 8
DUO = 1024  # psum staging width for ACT (2 m-chunks)

F32 = mybir.dt.float32
F32R = mybir.dt.float32r
F16 = mybir.dt.float16


def build_nc():
    nc = bacc.Bacc("TRN2", target_bir_lowering=False, debug=False)
    q_d = nc.dram_tensor("q", [H_PER_CORE, N, D], F32, kind="ExternalInput").ap()
    k_d = nc.dram_tensor("k", [H_PER_CORE, N, D], F32, kind="ExternalInput").ap()
    v_d = nc.dram_tensor("v", [H_PER_CORE, N, D], F32, kind="ExternalInput").ap()
    o_d = nc.dram_tensor("out", [H_PER_CORE, N, D], F32, kind="ExternalOutput").ap()

    with tile.TileContext(nc) as tc, ExitStack() as ctx:
        nat = ctx.enter_context(tc.tile_pool(name="nat", bufs=3))
        qt_p = ctx.enter_context(tc.tile_pool(name="qt", bufs=1))
        kt_p = ctx.enter_context(tc.tile_pool(name="kt", bufs=1))
        vp_p = ctx.enter_context(tc.tile_pool(name="vp", bufs=1))
        exp_p = ctx.enter_context(tc.tile_pool(name="exp", bufs=2))
        osb_p = ctx.enter_context(tc.tile_pool(name="osb", bufs=1))
        small = ctx.enter_context(tc.tile_pool(name="small", bufs=4))
        const_p = ctx.enter_context(tc.tile_pool(name="const", bufs=1))
        ps_mm1 = ctx.enter_context(tc.tile_pool(name="ps1", bufs=2, space="PSUM"))
        ps_mm2 = ctx.enter_context(tc.tile_pool(name="ps2", bufs=2, space="PSUM"))
        ps_t = ctx.enter_context(tc.tile_pool(name="pst", bufs=2, space="PSUM"))

        ident = const_p.tile([128, 128], F32)
        make_identity(nc, ident[:])

        nats = {}  # (h, name) -> tile, loads emitted ahead

        def load_head(h):
            for name, src in (("q", q_d), ("k", k_d), ("v", v_d)):
                t = nat.tile([128, NT * 128], F32, tag="nat")
                nc.sync.dma_start(
                    t[:].rearrange("p (t d) -> p t d", t=NT),
                    src[h].rearrange("(t p) d -> p t d", p=128),
                )
                nats[(h, name)] = t

        load_head(0)

        for h in range(H_PER_CORE):
            q_nat = nats.pop((h, "q"))
            k_nat = nats.pop((h, "k"))
            v_nat = nats.pop((h, "v"))

            # Transpose Q,K into [d, seq] layout via PE. Tiles are float32r
            # (the rounded fp32 the PE consumes at full rate) — the DVE
            # PSUM->SBUF copy performs the rounding.
            qt = qt_p.tile([128, N], F32R, tag="qt")
            kt = kt_p.tile([128, N], F32R, tag="kt")
            for t in range(NT):
                sl = slice(t * 128, (t + 1) * 128)
                pq = ps_t.tile([128, 128], F32, tag="pst")
                nc.tensor.transpose(pq[:], q_nat[:, sl], ident[:])
                nc.vector.tensor_copy(qt[:, sl], pq[:])
                pk = ps_t.tile([128, 128], F32, tag="pst")
                nc.tensor.transpose(pk[:], k_nat[:, sl], ident[:])
                nc.vector.tensor_copy(kt[:, sl], pk[:])

            # V (natural layout) cast to fp16, augmented with a ones column
            # per m-chunk: vplus chunk mc = [V_mc | 1] with 129 columns.
            vplus = vp_p.tile([128, NT * 129], F16, tag="vp")
            vp3 = vplus[:].rearrange("p (t c) -> p t c", c=129)
            nc.vector.tensor_copy(
                vp3[:, :, 0:128],
                v_nat[:].rearrange("p (t d) -> p t d", t=NT),
            )
            nc.gpsimd.memset(vp3[:, :, 128:129], 1.0)

            if h + 1 < H_PER_CORE:
                load_head(h + 1)  # prefetch next head while computing

            osb = osb_p.tile([128, NT * 128], F32, tag="osb")

            for nt in range(N_NTILES):
                qsl = slice(nt * N_TILE, (nt + 1) * N_TILE)
                # mm1 + exp: S^T chunks for all 32 m-chunks of this n-tile.
                expt = exp_p.tile([128, NT * N_TILE], F16, tag="exp")
                for duo in range(NT // 2):
                    ps = ps_mm1.tile([128, DUO], F32, tag="ps1")
                    for j in range(2):
                        mc = duo * 2 + j
                        nc.tensor.matmul(
                            ps[:, j * N_TILE : (j + 1) * N_TILE],
                            kt[:, mc * 128 : (mc + 1) * 128],
                            qt[:, qsl],
                            start=True,
                            stop=True,
                        )
                    nc.scalar.activation(
                        expt[:, duo * DUO : (duo + 1) * DUO],
                        ps[:],
                        mybir.ActivationFunctionType.Exp,
                        scale=SCALE,
                    )
                # mm2: per query subtile accumulate over m-chunks.
                for qs in range(N_TILE // 128):
                    po = ps_mm2.tile([128, 129], F32, tag="ps2")
                    for mc in range(NT):
                        base = mc * N_TILE + qs * 128
                        nc.tensor.matmul(
                            po[:],
                            expt[:, base : base + 128],
                            vplus[:, mc * 129 : (mc + 1) * 129],
                            start=(mc == 0),
                            stop=(mc == NT - 1),
                        )
                    rcp = small.tile([128, 1], F32, tag="rcp")
                    nc.vector.reciprocal(rcp[:], po[:, 128:129])
                    oc = (nt * (N_TILE // 128) + qs) * 128
                    nc.vector.tensor_scalar_mul(
                        osb[:, oc : oc + 128], po[:, 0:128], rcp[:]
                    )

            nc.sync.dma_start(
                o_d[h].rearrange("(t p) d -> p t d", p=128),
                osb[:].rearrange("p (t d) -> p t d", t=NT),
            )

    nc.finalize()
    return nc


_NC_CACHE = None


def _get_nc():
    global _NC_CACHE
    if _NC_CACHE is None:
        _NC_CACHE = build_nc()
    return _NC_CACHE


def run(q, k, v, **spmd_kwargs):
    nc = _get_nc()
    in_maps = [
        {
            "q": np.ascontiguousarray(q[i * H_PER_CORE : (i + 1) * H_PER_CORE]),
            "k": np.ascontiguousarray(k[i * H_PER_CORE : (i + 1) * H_PER_CORE]),
            "v": np.ascontiguousarray(v[i * H_PER_CORE : (i + 1) * H_PER_CORE]),
        }
        for i in range(N_CORES)
    ]
    res = run_bass_kernel_spmd(nc, in_maps, list(range(N_CORES)), **spmd_kwargs)
    out = np.concatenate([res.results[i]["out"] for i in range(N_CORES)], axis=0)
    return out.astype(np.float32), res


def kernel(q, k, v):
    q = np.asarray(q, dtype=np.float32)
    k = np.asarray(k, dtype=np.float32)
    v = np.asarray(v, dtype=np.float32)
    out, _ = run(q, k, v)
    return out
